# revision 4
# baseline (speedup 1.0000x reference)
"""AttentionBlock (GroupNorm -> 1x1 qkv conv -> spatial attention with
softmax over the last width axis -> 1x1 proj conv -> residual) on 8
Trainium2 NeuronCores, data-parallel over the batch.

v2: split-fp8 (e4m3 hi+lo) DoubleRow matmuls for the w / scores / proj
1x1-conv paths and an fp8 DoubleRow select-matmul for the softmax-
denominator partition reduction; the attention interior (E, A, v^T, h)
stays bf16. DoubleRow processes two 128-deep contraction tiles per
instruction at 0.5 cycles per output row (4x bf16 MAC throughput), so a
512-deep bf16 contraction (4 matmuls, 4*F cycles) becomes hi*hi + hi*lo
+ lo*hi (3 pairs, 1.5*F cycles) at ~bf16 accuracy: each fp8 tensor is
pre-scaled into e4m3's normal range (scales folded into host weights
and the ACT scale slots, so they are free), hi = round_fp8(psum), lo =
round_fp8(psum - hi) via one ACT copy + one DVE subtract straight from
psum. Host folds: scores Gram form S = n^T (Wq'^T Wk') n (drops the
separate q/k projections), value path (proj_w Wv)(n A), C**-0.25 into
the Gram matrix, v-bias into the proj bias. The softmax denominator D
sums j-partition groups of E via an fp8 select matmul (entries 1/4,
giving A at x4 scale) over an fp8 cast of E made on Pool; exp uses a
folded logit shift so E fits e4m3's 240 max. GroupNorm runs as v1:
channel bn_stats + tiny PE select-matmul group combine + quake-seed
Newton rsqrt.

Per-sample PE cost (cycles at 2.4GHz): w 12288, S 24576, v^T transpose
4096, D 2048, h 32768 (bf16: fp8 A fails the accuracy gate - softmax
logit noise amplifies), proj 12288 vs v1's ~112k. The pipeline
schedule: scores/exp(s) -> stats(s+1) -> softmax(s) with proj(s-1)
matmuls interleaved between the D matmuls -> h(s) -> normalize(s+1) +
w/u/vT(s+1), so the Pool-side E8 casts and normalize chain of s+1 hide
under h(s) and the softmax tail hides under proj(s-1)."""

import os
from contextlib import ExitStack

import numpy as np

B, C, H, W = 32, 512, 32, 32
HW = H * W            # 1024
G = 32                # groupnorm groups
GS = C // G           # 16 channels per group
NCORES = 8
BS = B // NCORES      # 4 samples per core
EPS = 1e-5
P = 128
CT = C // P           # 4 channel tiles
IJT = HW // P         # 8 key-pixel tiles
NF = 512              # matmul moving free dim
NH = HW // NF         # 2

MTS = 128.0           # host scale on the Gram matrix -> w/S psum scale
PTS = 32.0            # host scale on the proj matrix
AS = 4.0              # A scale (select-matrix entries 1/AS)
SHIFT = 1.25          # logit shift so exp fits fp8 e4m3 (max 240)

# "fp8" (default): D reduction via fp8 DoubleRow over an fp8 cast of E
# "bf16": D reduction via bf16 select matmuls directly on E (no cast)
D_MODE = os.environ.get("ATTN_V2_D", "fp8")
MM_MODE = f"v2-{D_MODE}"

_cache: dict = {}


def _build(d_mode: str):
    import concourse.bass as bass
    import concourse.tile as tile
    from concourse import bacc, mybir

    dt = mybir.dt
    AF = mybir.ActivationFunctionType
    ALU = mybir.AluOpType
    PM = mybir.MatmulPerfMode
    f32 = dt.float32
    bf16 = dt.bfloat16
    f8 = dt.float8e4
    DR = PM.DoubleRow
    fp8_d = d_mode == "fp8"

    nc = bacc.Bacc("TRN2", target_bir_lowering=False, debug=False,
                   dynamic_dma_scratch_size=8192)

    x_d = nc.dram_tensor("x", [BS, C, HW], bf16, kind="ExternalInput").ap()
    mh_d = nc.dram_tensor("mh", [P, CT, C], f8, kind="ExternalInput").ap()
    ml_d = nc.dram_tensor("ml", [P, CT, C], f8, kind="ExternalInput").ap()
    id_d = nc.dram_tensor("ident", [P, P], bf16, kind="ExternalInput").ap()
    vu_d = nc.dram_tensor("vu", [P, CT, 1], bf16, kind="ExternalInput").ap()
    ph_d = nc.dram_tensor("pth", [P, CT, C], f8, kind="ExternalInput").ap()
    pl_d = nc.dram_tensor("ptl", [P, CT, C], f8, kind="ExternalInput").ap()
    pb_d = nc.dram_tensor("pb", [P, CT], f32, kind="ExternalInput").ap()
    gw_d = nc.dram_tensor("gw", [P, CT], f32, kind="ExternalInput").ap()
    gb_d = nc.dram_tensor("gb", [P, CT], f32, kind="ExternalInput").ap()
    sg_d = nc.dram_tensor("selg", [P, 8], f32, kind="ExternalInput").ap()
    ss_dt = f8 if fp8_d else bf16
    ss_d = nc.dram_tensor("ssum", [P, IJT, G], ss_dt, kind="ExternalInput").ap()
    out_d = nc.dram_tensor("out", [BS, C, HW], f32, kind="ExternalOutput").ap()

    with tile.TileContext(nc) as tc, ExitStack() as ctx:
        singles = ctx.enter_context(tc.tile_pool(name="singles", bufs=1))
        # pmm tiles are 2-bank [P, NH, NF] pairs so ACT psum->sbuf ops run
        # 1024-wide
        pmm = ctx.enter_context(tc.tile_pool(name="pmm", bufs=2, space="PSUM"))
        pdp = ctx.enter_context(tc.tile_pool(name="pdp", bufs=3, space="PSUM"))
        pup = ctx.enter_context(tc.tile_pool(name="pup", bufs=1, space="PSUM"))

        def pbc(base, rep):
            # partition-broadcast source AP: replicate each source partition
            # `rep` times (destination iterates partitions major)
            base = base.opt(keep_dims={0})
            ap = [d for d in base.ap[1:] if d[1] > 1] or [[1, 1]]
            return bass.AP(
                tensor=base.tensor, offset=base.offset,
                ap=[base.ap[0], [0, rep], *ap],
            )

        # startup DMA priority: xt(0) first (gates GN stats), then the GN
        # smalls, mh/ml (gate w(0)), then ident/vu/ss; pth/ptl deferred
        xtp = ctx.enter_context(tc.tile_pool(name="xtp", bufs=4))
        xts = {}
        xts[0] = xtp.tile([P, CT, HW], bf16, tag="xt", name="xt0")
        xv0 = x_d[0].rearrange("(q p) f -> p q f", p=P)
        dma_engs = (nc.sync, nc.scalar, nc.gpsimd)
        for h2 in range(2 * CT):
            q, sub = divmod(h2, 2)
            dma_engs[h2 % 3].dma_start(
                xts[0][:, q, sub * NF : (sub + 1) * NF],
                xv0[:, q, sub * NF : (sub + 1) * NF],
            )
        selg_sb = singles.tile([P, 8], f32)
        nc.scalar.dma_start(selg_sb, sg_d)
        gw_sb = singles.tile([P, CT], f32)
        nc.gpsimd.dma_start(gw_sb, gw_d)
        gb_sb = singles.tile([P, CT], f32)
        nc.gpsimd.dma_start(gb_sb, gb_d)
        pb_sb = singles.tile([P, CT], f32)
        nc.gpsimd.dma_start(pb_sb, pb_d)
        mh_sb = singles.tile([P, CT, C], f8)
        for ot in range(CT):
            nc.sync.dma_start(mh_sb[:, :, ot * P : (ot + 1) * P],
                              mh_d[:, :, ot * P : (ot + 1) * P])
        ml_sb = singles.tile([P, CT, C], f8)
        for ot in range(CT):
            nc.sync.dma_start(ml_sb[:, :, ot * P : (ot + 1) * P],
                              ml_d[:, :, ot * P : (ot + 1) * P])
        ident_sb = singles.tile([P, P], bf16)
        nc.sync.dma_start(ident_sb, id_d)
        vu_sb = singles.tile([P, CT, 1], bf16)
        nc.scalar.dma_start(vu_sb, vu_d)
        ss_sb = singles.tile([P, IJT, G], ss_dt)
        nc.gpsimd.dma_start(ss_sb, ss_d)

        small = ctx.enter_context(tc.tile_pool(name="small", bufs=1))
        stp = ctx.enter_context(tc.tile_pool(name="stp", bufs=4))
        epsb = singles.tile([P, 1], f32)
        nc.vector.memset(epsb, EPS)
        # warm the Exp ACT table set while the first DMAs run
        actwarm = singles.tile([P, 1], f32)
        nc.scalar.activation(out=actwarm, in_=epsb, func=AF.Exp)
        magic = singles.tile([8, CT, 1], dt.int32)
        nc.vector.memset(magic, 0x5F3759DF)
        shiftb = singles.tile([P, 1], f32)
        nc.vector.memset(shiftb, -SHIFT)
        gst = singles.tile([8, BS * CT, 2], f32)
        scv = singles.tile([P, BS * CT], f32)
        tcv = singles.tile([P, BS * CT], f32)

        def emit_stats(s):
            """Channel bn_stats on xt(s) -> group combine on PE -> per-channel
            GN scale/offset columns scv/tcv[:, s*CT..]."""
            xt = xts[s]
            for q in range(CT):
                stq = stp.tile([P, 2, 6], f32, tag="stq")
                for sub in range(2):
                    nc.vector.bn_stats(
                        out=stq[:, sub, :], in_=xt[:, q, sub * 512 : (sub + 1) * 512]
                    )
                mvq = stp.tile([P, 2], f32, tag="mvq")
                nc.vector.bn_aggr(out=mvq, in_=stq)
                exq = stp.tile([P, 2], f32, tag="exq")
                nc.vector.tensor_copy(out=exq[:, 0:1], in_=mvq[:, 0:1])
                nc.vector.tensor_scalar(
                    exq[:, 1:2], mvq[:, 0:1], mvq[:, 0:1], mvq[:, 1:2],
                    op0=ALU.mult, op1=ALU.add,
                )
                pg = pdp.tile([8, 2], f32, tag="pd")
                nc.tensor.matmul(pg, lhsT=selg_sb, rhs=exq, start=True, stop=True)
                nc.vector.tensor_copy(out=gst[0:8, s * CT + q, :], in_=pg)
            gm = gst[0:8, s * CT : (s + 1) * CT, 0:1]
            gx2 = gst[0:8, s * CT : (s + 1) * CT, 1:2]
            # group combine + Newton rsqrt on Pool: DVE must stay clear for
            # the previous sample's softmax chain
            veng = nc.gpsimd
            gv = stp.tile([8, CT, 1], f32, tag="gv")
            veng.tensor_tensor(gv, gm, gm, ALU.mult)
            veng.tensor_tensor(gv, gx2, gv, ALU.subtract)
            veng.tensor_scalar(gv, gv, EPS, None, op0=ALU.add)
            i32 = dt.int32
            yb = stp.tile([8, CT, 1], f32, tag="yb")
            nc.vector.tensor_scalar(
                yb.bitcast(i32), gv.bitcast(i32), 1, None,
                op0=ALU.arith_shift_right,
            )
            nc.vector.tensor_tensor(
                yb.bitcast(i32), magic, yb.bitcast(i32), ALU.subtract
            )
            hh = stp.tile([8, CT, 1], f32, tag="hh")
            veng.tensor_scalar(hh, gv, 0.5, None, op0=ALU.mult)
            ttn = stp.tile([8, CT, 1], f32, tag="ttn")
            for _ in range(2):
                veng.tensor_tensor(ttn, yb, yb, ALU.mult)
                veng.tensor_tensor(ttn, hh, ttn, ALU.mult)
                veng.tensor_scalar(
                    ttn, ttn, -1.0, 1.5, op0=ALU.mult, op1=ALU.add
                )
                veng.tensor_tensor(yb, yb, ttn, ALU.mult)
            gv = yb
            rstdb = stp.tile([P, CT], f32, tag="rstdb")
            with tc.high_priority(offset=1 << 20):
                nc.sync.dma_start(
                    rstdb.opt(keep_dims={0}), pbc(gv[0:8, :, 0], 16)
                )
            gmt = stp.tile([8, CT, 1], f32, tag="gmt")
            veng.tensor_copy(out=gmt, in_=gm)
            gmb = stp.tile([P, CT], f32, tag="gmb")
            with tc.high_priority(offset=1 << 20):
                nc.sync.dma_start(
                    gmb.opt(keep_dims={0}), pbc(gmt[0:8, :, 0], 16)
                )
            cs = scv[:, s * CT : (s + 1) * CT]
            veng.tensor_tensor(cs, gw_sb, rstdb, ALU.mult)
            tmpb = stp.tile([P, CT], f32, tag="tmpb")
            veng.tensor_tensor(tmpb, gmb, cs, ALU.mult)
            veng.tensor_tensor(
                tcv[:, s * CT : (s + 1) * CT], gb_sb, tmpb, ALU.subtract
            )

        emit_stats(0)

        ptmp = ctx.enter_context(tc.tile_pool(name="ptmp", bufs=4))
        bigs = ctx.enter_context(tc.tile_pool(name="bigs", bufs=1))
        rbp = ctx.enter_context(tc.tile_pool(name="rbp", bufs=8))

        # ---- per-sample attention ----
        nts, nhs, nls = {}, {}, {}

        def emit_normalize(s):
            # t = GN(x) in bf16, then nh = fp8(t) and nl = fp8(t - nh).
            # t/nh on Pool, nl on DVE (steady state); sample 0 alternates
            # engines per 512-wide half on the startup critical path.
            nt = bigs.tile([P, CT, HW], bf16, tag="nt", bufs=2, name=f"nt{s}")
            nh = bigs.tile([P, CT, HW], f8, tag="nh", bufs=2, name=f"nh{s}")
            nl = bigs.tile([P, CT, HW], f8, tag="nl", bufs=2, name=f"nl{s}")
            nts[s], nhs[s], nls[s] = nt, nh, nl
            if s == 0:
                for h2 in range(2 * CT):
                    q, sub = divmod(h2, 2)
                    e0 = nc.vector if h2 % 2 == 0 else nc.gpsimd
                    e1 = nc.gpsimd if h2 % 2 == 0 else nc.vector
                    sl = slice(sub * NF, (sub + 1) * NF)
                    e0.tensor_scalar(
                        nt[:, q, sl], xts[s][:, q, sl],
                        scv[:, s * CT + q : s * CT + q + 1],
                        tcv[:, s * CT + q : s * CT + q + 1],
                        op0=ALU.mult, op1=ALU.add,
                    )
                    e1.tensor_copy(out=nh[:, q, sl], in_=nt[:, q, sl])
                    e0.tensor_tensor(nl[:, q, sl], nt[:, q, sl], nh[:, q, sl],
                                     ALU.subtract)
                return
            for q in range(CT):
                nc.gpsimd.tensor_scalar(
                    nt[:, q], xts[s][:, q],
                    scv[:, s * CT + q : s * CT + q + 1],
                    tcv[:, s * CT + q : s * CT + q + 1],
                    op0=ALU.mult, op1=ALU.add,
                )
                nc.gpsimd.tensor_copy(out=nh[:, q], in_=nt[:, q])
                nc.vector.tensor_tensor(nl[:, q], nt[:, q], nh[:, q],
                                        ALU.subtract)

        emit_normalize(0)
        pth_sb = singles.tile([P, CT, C], f8)
        ptl_sb = singles.tile([P, CT, C], f8)
        whs, wls, vts, ess, e8s, uss, hhs, hls = {}, {}, {}, {}, {}, {}, {}, {}

        def dr6(ps_n, lhi, llo, rhi, rlo, ocols, ncols):
            # hi*hi + hi*lo + lo*hi via 6 DoubleRow matmuls (2 k-tile pairs
            # each) accumulating into one psum
            idx = 0
            for L, R in ((lhi, rhi), (lhi, rlo), (llo, rhi)):
                for kp in range(CT // 2):
                    nc.tensor.matmul(
                        ps_n,
                        lhsT=L[:, 2 * kp : 2 * kp + 2, ocols],
                        rhs=R[:, 2 * kp : 2 * kp + 2, ncols],
                        start=(idx == 0), stop=(idx == 5),
                        perf_mode=DR,
                    )
                    idx += 1

        def w_groups(s):
            # w = (Wq'^T Wk') n at xMTS: split-fp8 DoubleRow; psum lands at
            # the fp8 target scale so hi = ACT copy, lo = DVE psum subtract
            nh, nl = nhs[s], nls[s]
            wh = bigs.tile([P, CT, HW], f8, tag="wh", bufs=2, name=f"wh{s}")
            wl = bigs.tile([P, CT, HW], f8, tag="wl", bufs=2, name=f"wl{s}")
            whs[s], wls[s] = wh, wl

            def w_one(ot):
                ps = pmm.tile([P, NH, NF], f32, tag="mm")
                for n in range(NH):
                    dr6(ps[:, n], mh_sb, ml_sb, nh, nl,
                        slice(ot * P, (ot + 1) * P),
                        slice(n * NF, (n + 1) * NF))
                nc.scalar.activation(out=wh[:, ot], in_=ps, func=AF.Copy)
                nc.vector.tensor_tensor(wl[:, ot], ps, wh[:, ot], ALU.subtract)

            return [lambda ot=ot: w_one(ot) for ot in range(CT)]

        def emit_u(s):
            # u[ij] = bq'^T k'[:,ij] - SHIFT, the per-partition exp bias
            usb = small.tile([P, IJT], f32, tag="u", bufs=2, name=f"u{s}")
            uss[s] = usb
            nt = nts[s]
            for t in range(IJT):
                pu = pup.tile([P, 1], f32, tag="pu")
                for k in range(CT):
                    nc.tensor.matmul(
                        pu,
                        lhsT=nt[:, k, t * P : (t + 1) * P],
                        rhs=vu_sb[:, k],
                        start=(k == 0), stop=(k == CT - 1),
                    )
                nc.scalar.activation(
                    out=usb[:, t : t + 1], in_=pu, func=AF.Identity,
                    bias=shiftb,
                )

        def vt_groups(s):
            # n^T[ij, c] via PE transpose of the bf16 t
            nt = nts[s]
            vtsb = bigs.tile([P, IJT, C], bf16, tag="vt", bufs=2, name=f"ntt{s}")
            vts[s] = vtsb

            def vt_one(t2):
                ps = pmm.tile([P, 2, NF], bf16, tag="mm")
                for i2 in range(2):
                    t = 2 * t2 + i2
                    for k in range(CT):
                        nc.tensor.transpose(
                            ps[:, i2, k * P : (k + 1) * P],
                            nt[:, k, t * P : (t + 1) * P],
                            ident_sb,
                        )
                nc.scalar.activation(
                    out=vtsb[:, 2 * t2 : 2 * t2 + 2], in_=ps, func=AF.Identity
                )

            return [lambda t2=t2: vt_one(t2) for t2 in range(IJT // 2)]

        def emit_scores_exp(s):
            # S^T[ij, hw] at xMTS via split-fp8 DoubleRow; E = exp(S + u -
            # SHIFT) in bf16; E8 = fp8 cast on Pool for the D reduction
            wh, wl, nh, nl, usb = whs[s], wls[s], nhs[s], nls[s], uss[s]
            esb = bigs.tile([P, IJT, HW], bf16, tag="E", bufs=2, name=f"E{s}")
            ess[s] = esb
            if fp8_d:
                e8 = bigs.tile([P, IJT, HW], f8, tag="E8", bufs=2, name=f"E8{s}")
                e8s[s] = e8
            for t in range(IJT):
                ps = pmm.tile([P, NH, NF], f32, tag="mm")
                for n in range(NH):
                    dr6(ps[:, n], wh, wl, nh, nl,
                        slice(t * P, (t + 1) * P),
                        slice(n * NF, (n + 1) * NF))
                nc.scalar.activation(
                    out=esb[:, t], in_=ps, func=AF.Exp,
                    bias=usb[:, t : t + 1], scale=1.0 / MTS,
                )
                if fp8_d:
                    nc.gpsimd.tensor_copy(out=e8s[s][:, t], in_=esb[:, t])

        def emit_softmax(s, fillers=()):
            # per-(i,hw) denominators D via select matmuls (fp8 DoubleRow on
            # E8, or bf16 on E), R = AS/D via DVE fast-recip + Newton into the
            # bf16 broadcast source, A = E * broadcast(R) on DVE in place.
            # `fillers` are PE closures (prev sample's proj) interleaved
            # between the D matmuls, which pace at the exp/cast stream.
            from concourse.dve_ops import RECIPROCAL_APPROX_NR

            esb = ess[s]
            fillers = list(fillers)
            emitted = 0
            rsc = small.tile([G, HW], f32, tag="rsc")
            rrb = small.tile([G, HW], bf16, tag="rrb")
            nd = IJT // 2 if fp8_d else IJT
            for n in range(NH):
                pd = pdp.tile([G, NF], f32, tag="pd")
                for t in range(nd):
                    if fp8_d:
                        nc.tensor.matmul(
                            pd,
                            lhsT=ss_sb[:, 2 * t : 2 * t + 2, :],
                            rhs=e8s[s][:, 2 * t : 2 * t + 2,
                                       n * NF : (n + 1) * NF],
                            start=(t == 0), stop=(t == nd - 1),
                            perf_mode=DR,
                        )
                    else:
                        nc.tensor.matmul(
                            pd,
                            lhsT=ss_sb[:, t, :],
                            rhs=esb[:, t, n * NF : (n + 1) * NF],
                            start=(t == 0), stop=(t == nd - 1),
                        )
                    done = n * nd + t + 1
                    want = len(fillers) * done // (NH * nd)
                    while emitted < want:
                        fillers[emitted]()
                        emitted += 1
                nc.vector.reciprocal_approx_fast(
                    out=rsc[:, n * NF : (n + 1) * NF], in_=pd
                )
                nc.vector._custom_dve(
                    RECIPROCAL_APPROX_NR,
                    out=rrb[:, n * NF : (n + 1) * NF],
                    in0=pd,
                    in1=rsc[:, n * NF : (n + 1) * NF],
                    s0=2.0,
                )
            for f in fillers[emitted:]:
                f()
            for t in range(IJT):
                rbt = rbp.tile([P, HW], bf16, tag="rb")
                (nc.sync, nc.gpsimd, nc.scalar)[t % 3].dma_start(
                    rbt, pbc(rrb[4 * t : 4 * t + 4, :], 32)
                )
                nc.vector.tensor_tensor(esb[:, t], esb[:, t], rbt, ALU.mult)

        def emit_h(s):
            # h[c, hw] at xAS = sum_ij v^T[ij,c] A^T[ij,hw] in bf16, split to
            # fp8 hi/lo from psum for the split-fp8 proj
            vtsb, esb = vts[s], ess[s]
            hh = bigs.tile([P, CT, HW], f8, tag="hh", bufs=2, name=f"hh{s}")
            hl = bigs.tile([P, CT, HW], f8, tag="hl", bufs=2, name=f"hl{s}")
            hhs[s], hls[s] = hh, hl
            for ct in range(CT):
                ps = pmm.tile([P, NH, NF], f32, tag="mm")
                for n in range(NH):
                    for t in range(IJT):
                        nc.tensor.matmul(
                            ps[:, n],
                            lhsT=vtsb[:, t, ct * P : (ct + 1) * P],
                            rhs=esb[:, t, n * NF : (n + 1) * NF],
                            start=(t == 0), stop=(t == IJT - 1),
                        )
                nc.scalar.activation(out=hh[:, ct], in_=ps, func=AF.Copy)
                nc.vector.tensor_tensor(hl[:, ct], ps, hh[:, ct], ALU.subtract)

        def proj_groups(s):
            # split-fp8 proj + bias + residual, then store
            hh, hl, xt = hhs[s], hls[s], xts[s]
            ov = out_d[s].rearrange("(q p) f -> p q f", p=P)

            def p_one(ot):
                ps = pmm.tile([P, NH, NF], f32, tag="mm")
                for n in range(NH):
                    dr6(ps[:, n], pth_sb, ptl_sb, hh, hl,
                        slice(ot * P, (ot + 1) * P),
                        slice(n * NF, (n + 1) * NF))
                tmp = ptmp.tile([P, HW], f32, tag="pt")
                nc.scalar.activation(
                    out=tmp, in_=ps, func=AF.Identity,
                    bias=pb_sb[:, ot : ot + 1], scale=1.0 / (PTS * AS),
                )
                nc.vector.tensor_tensor(tmp, tmp, xt[:, ot], ALU.add)
                nc.sync.dma_start(ov[:, ot], tmp)

            return [lambda ot=ot: p_one(ot) for ot in range(CT)]

        def emit_proj_fine(s):
            # final sample: 512-wide residuals and stores so the tail drains
            # with finer overlap
            hh, hl, xt = hhs[s], hls[s], xts[s]
            ov = out_d[s].rearrange("(q p) f -> p q f", p=P)
            for ot in range(CT):
                ps = pmm.tile([P, NH, NF], f32, tag="mm")
                for n in range(NH):
                    dr6(ps[:, n], pth_sb, ptl_sb, hh, hl,
                        slice(ot * P, (ot + 1) * P),
                        slice(n * NF, (n + 1) * NF))
                tmp = ptmp.tile([P, HW], f32, tag="pt")
                nc.scalar.activation(
                    out=tmp, in_=ps, func=AF.Identity,
                    bias=pb_sb[:, ot : ot + 1], scale=1.0 / (PTS * AS),
                )
                for n in range(NH):
                    sl = slice(n * NF, (n + 1) * NF)
                    nc.vector.tensor_tensor(
                        tmp[:, sl], tmp[:, sl], xt[:, ot, sl], ALU.add
                    )
                    (nc.sync if n == 0 else nc.scalar).dma_start(
                        ov[:, ot, sl], tmp[:, sl]
                    )

        # software pipeline: scores/exp(s) -> stats(s+1) -> softmax(s) with
        # proj(s-1) interleaved between D matmuls -> h(s) -> normalize(s+1)
        # and w/u/vT(s+1) (emitted after h so the PE does not stall on the
        # s+1 normalize chain, which runs behind the E8 casts on Pool)
        for f in w_groups(0):
            f()
        emit_u(0)
        for f in vt_groups(0):
            f()
        for s in range(BS):
            emit_scores_exp(s)
            if s == 0:
                # deferred + chunked: needed only by proj(0) much later
                for ot in range(CT):
                    nc.scalar.dma_start(
                        pth_sb[:, :, ot * P : (ot + 1) * P],
                        ph_d[:, :, ot * P : (ot + 1) * P],
                    )
                    nc.scalar.dma_start(
                        ptl_sb[:, :, ot * P : (ot + 1) * P],
                        pl_d[:, :, ot * P : (ot + 1) * P],
                    )
            if s + 1 < BS:
                xts[s + 1] = xtp.tile([P, CT, HW], bf16, tag="xt",
                                      name=f"xt{s + 1}")
                xvn = x_d[s + 1].rearrange("(q p) f -> p q f", p=P)
                for q in range(CT):
                    nc.sync.dma_start(xts[s + 1][:, q], xvn[:, q])
                emit_stats(s + 1)
            emit_softmax(s, fillers=proj_groups(s - 1) if s >= 1 else ())
            emit_h(s)
            if s + 1 < BS:
                emit_normalize(s + 1)
                for f in w_groups(s + 1):
                    f()
                emit_u(s + 1)
                for f in vt_groups(s + 1):
                    f()
        emit_proj_fine(BS - 1)

    nc.compile()
    return nc


def _prep_inputs(x, gn_w, gn_b, qkv_w, qkv_b, proj_w, proj_b):
    import ml_dtypes

    f8 = ml_dtypes.float8_e4m3
    bfnp = ml_dtypes.bfloat16

    x = np.asarray(x, dtype=np.float32)
    gn_w = np.asarray(gn_w, dtype=np.float32)
    gn_b = np.asarray(gn_b, dtype=np.float32)
    qkv_w = np.asarray(qkv_w, dtype=np.float32)
    qkv_b = np.asarray(qkv_b, dtype=np.float32)
    proj_w = np.asarray(proj_w, dtype=np.float32)
    proj_b = np.asarray(proj_b, dtype=np.float32)

    s4 = np.float32(float(C) ** -0.25)
    Wq = (qkv_w[:C] * s4).astype(np.float64)
    Wk = (qkv_w[C : 2 * C] * s4).astype(np.float64)
    bq = (qkv_b[:C] * s4).astype(np.float64)
    # Gram fold: S = n^T (Wq^T Wk) n + (Wk^T bq).n_ij, at xMTS for fp8
    mt = (Wk.T @ Wq).astype(np.float32) * np.float32(MTS)   # [C, C] c_in x o
    mh = mt.astype(f8)
    ml_ = (mt - mh.astype(np.float32)).astype(f8)

    def karr(a):
        # [C, C] (c_in, o) -> [P, CT, C] with c_in = k*P + p
        return np.ascontiguousarray(a.reshape(CT, P, C).transpose(1, 0, 2))

    vu = np.ascontiguousarray(
        (Wk.T @ bq).astype(np.float32).reshape(CT, P).T[:, :, None]
    ).astype(bfnp)                                          # [P, CT, 1]
    Wv = qkv_w[2 * C :].astype(np.float64)
    pt = ((proj_w.astype(np.float64) @ Wv).T.astype(np.float32)
          * np.float32(PTS))                                # [C, C] c_in x o
    pth = pt.astype(f8)
    ptl = (pt - pth.astype(np.float32)).astype(f8)
    ident = np.eye(P, dtype=np.float32).astype(bfnp)
    vb = qkv_b[2 * C :]
    pb = np.ascontiguousarray(
        (proj_b + np.float32(H) * (proj_w @ vb)).reshape(CT, P).T
    )                                                       # [P, CT]
    gw = np.ascontiguousarray(gn_w.reshape(CT, P).T)
    gb = np.ascontiguousarray(gn_b.reshape(CT, P).T)
    selg = np.zeros((P, 8), dtype=np.float32)
    selg[np.arange(P), np.arange(P) // 16] = 1.0 / 16.0
    ss = np.zeros((P, IJT, G), dtype=np.float32)
    for t in range(IJT):
        for p in range(P):
            ss[p, t, 4 * t + p // G] = 1.0 / AS
    ss = ss.astype(f8 if D_MODE == "fp8" else bfnp)
    shared = {
        "mh": karr(mh), "ml": karr(ml_), "vu": vu,
        "pth": karr(pth), "ptl": karr(ptl), "pb": pb,
        "ident": ident, "gw": gw, "gb": gb, "ssum": ss, "selg": selg,
    }
    in_maps = []
    for c in range(NCORES):
        m = dict(shared)
        m["x"] = np.ascontiguousarray(
            x[c * BS : (c + 1) * BS].reshape(BS, C, HW)
        ).astype(bfnp)
        in_maps.append(m)
    return in_maps


def run(inputs: dict, trace: bool = False, n_cores: int = NCORES):
    """Build (cached), run on hardware, return BassKernelResults."""
    from concourse.bass_utils import run_bass_kernel_spmd

    key = MM_MODE
    if key not in _cache:
        _cache[key] = _build(D_MODE)
    nc = _cache[key]
    in_maps = _prep_inputs(**inputs)[:n_cores]
    res = run_bass_kernel_spmd(nc, in_maps, list(range(n_cores)), trace=trace)
    return res


def kernel(x, gn_w, gn_b, qkv_w, qkv_b, proj_w, proj_b) -> np.ndarray:
    res = run(dict(x=x, gn_w=gn_w, gn_b=gn_b, qkv_w=qkv_w, qkv_b=qkv_b,
                   proj_w=proj_w, proj_b=proj_b))
    out = np.concatenate(
        [res.results[c]["out"].reshape(BS, C, H, W) for c in range(NCORES)], axis=0
    )
    return out


# revision 50
# speedup vs baseline: 1.1790x; 1.1790x over previous
"""AttentionBlock (GroupNorm -> 1x1 qkv conv -> spatial attention with
softmax over the last width axis -> 1x1 proj conv -> residual) on 8
Trainium2 NeuronCores, data-parallel over the batch.

v2: split-fp8 (e4m3 hi+lo) DoubleRow matmuls for the w / scores / proj
1x1-conv paths and an fp8 DoubleRow select-matmul for the softmax-
denominator partition reduction; the attention interior (E, A, v^T, h)
stays bf16. DoubleRow processes two 128-deep contraction tiles per
instruction at 0.5 cycles per output row (4x bf16 MAC throughput), so a
512-deep bf16 contraction (4 matmuls, 4*F cycles) becomes hi*hi + hi*lo
+ lo*hi (3 pairs, 1.5*F cycles) at ~bf16 accuracy: each fp8 tensor is
pre-scaled into e4m3's normal range (scales folded into host weights
and the ACT scale slots, so they are free), hi = round_fp8(psum), lo =
round_fp8(psum - hi) via one ACT copy + one DVE subtract straight from
psum. Host folds: scores Gram form S = n^T (Wq'^T Wk') n (drops the
separate q/k projections), value path (proj_w Wv)(n A), C**-0.25 into
the Gram matrix, v-bias into the proj bias. The softmax denominator D
sums j-partition groups of E via an fp8 select matmul (entries 1/4,
giving A at x4 scale) over an fp8 cast of E made on Pool; exp uses a
folded logit shift so E fits e4m3's 240 max. GroupNorm runs as v1:
channel bn_stats + tiny PE select-matmul group combine + quake-seed
Newton rsqrt.

Per-sample PE cost (cycles at 2.4GHz): w 12288, S 24576, v^T transpose
4096, D 2048, h 32768 (bf16: fp8 A fails the accuracy gate - softmax
logit noise amplifies), proj 12288 vs v1's ~112k. The pipeline
schedule: scores/exp(s) -> stats(s+1) -> softmax(s) with proj(s-1)
matmuls interleaved between the D matmuls -> h(s) -> normalize(s+1) +
w/u/vT(s+1), so the Pool-side E8 casts and normalize chain of s+1 hide
under h(s) and the softmax tail hides under proj(s-1)."""

import os
from contextlib import ExitStack

import numpy as np

B, C, H, W = 32, 512, 32, 32
HW = H * W            # 1024
G = 32                # groupnorm groups
GS = C // G           # 16 channels per group
NCORES = 8
BS = B // NCORES      # 4 samples per core
EPS = 1e-5
P = 128
CT = C // P           # 4 channel tiles
IJT = HW // P         # 8 key-pixel tiles
NF = 512              # matmul moving free dim
NH = HW // NF         # 2

MTS = 128.0           # host scale on the Gram matrix -> w/S psum scale
PTS = 32.0            # host scale on the proj matrix
AS = 4.0              # A scale (select-matrix entries 1/AS)
SHIFT = 1.25          # logit shift so exp fits fp8 e4m3 (max 240)

# "fp8" (default): D reduction via fp8 DoubleRow over an fp8 cast of E
# "bf16": D reduction via bf16 select matmuls directly on E (no cast)
D_MODE = os.environ.get("ATTN_V2_D", "fp8")
MM_MODE = f"v2-{D_MODE}"

_cache: dict = {}


def _build(d_mode: str):
    import concourse.bass as bass
    import concourse.tile as tile
    from concourse import bacc, mybir

    dt = mybir.dt
    AF = mybir.ActivationFunctionType
    ALU = mybir.AluOpType
    PM = mybir.MatmulPerfMode
    f32 = dt.float32
    bf16 = dt.bfloat16
    f8 = dt.float8e4
    DR = PM.DoubleRow
    fp8_d = d_mode == "fp8"

    nc = bacc.Bacc("TRN2", target_bir_lowering=False, debug=False,
                   dynamic_dma_scratch_size=8192)

    x_d = nc.dram_tensor("x", [BS, C, HW], bf16, kind="ExternalInput").ap()
    mh_d = nc.dram_tensor("mh", [P, CT, C], f8, kind="ExternalInput").ap()
    ml_d = nc.dram_tensor("ml", [P, CT, C], f8, kind="ExternalInput").ap()
    id_d = nc.dram_tensor("ident", [P, P], bf16, kind="ExternalInput").ap()
    vu_d = nc.dram_tensor("vu", [P, CT, 1], bf16, kind="ExternalInput").ap()
    ph_d = nc.dram_tensor("pth", [P, CT, C], f8, kind="ExternalInput").ap()
    pl_d = nc.dram_tensor("ptl", [P, CT, C], f8, kind="ExternalInput").ap()
    pb_d = nc.dram_tensor("pb", [P, CT], f32, kind="ExternalInput").ap()
    gw_d = nc.dram_tensor("gw", [P, CT], f32, kind="ExternalInput").ap()
    gb_d = nc.dram_tensor("gb", [P, CT], f32, kind="ExternalInput").ap()
    sg_d = nc.dram_tensor("selg", [P, 8], f32, kind="ExternalInput").ap()
    ss_dt = f8 if fp8_d else bf16
    ss_d = nc.dram_tensor("ssum", [P, IJT, G], ss_dt, kind="ExternalInput").ap()
    out_d = nc.dram_tensor("out", [BS, C, HW], f32, kind="ExternalOutput").ap()

    with tile.TileContext(nc) as tc, ExitStack() as ctx:
        singles = ctx.enter_context(tc.tile_pool(name="singles", bufs=1))
        # pmm tiles are 2-bank [P, NH, NF] pairs so ACT psum->sbuf ops run
        # 1024-wide
        pmm = ctx.enter_context(tc.tile_pool(name="pmm", bufs=2, space="PSUM"))
        pdp = ctx.enter_context(tc.tile_pool(name="pdp", bufs=2, space="PSUM"))
        pgp = ctx.enter_context(tc.tile_pool(name="pgp", bufs=1, space="PSUM"))
        pup = ctx.enter_context(tc.tile_pool(name="pup", bufs=1, space="PSUM"))

        def pbc(base, rep):
            # partition-broadcast source AP: replicate each source partition
            # `rep` times (destination iterates partitions major)
            base = base.opt(keep_dims={0})
            ap = [d for d in base.ap[1:] if d[1] > 1] or [[1, 1]]
            return bass.AP(
                tensor=base.tensor, offset=base.offset,
                ap=[base.ap[0], [0, rep], *ap],
            )

        # startup DMA priority: xt(0) first (gates GN stats), then the GN
        # smalls, mh/ml (gate w(0)), then ident/vu/ss; pth/ptl deferred
        # The DMA transfers serialize through one DMA-engines resource, so
        # startup ORDER is everything: x0 (gates stats), the GN smalls
        # (gate the combine), then mh/ml (gate w(0)), ident/vu/ss. The
        # proj weights are deferred to the end of iteration 0. sync/scalar
        # queue issue is cheap; gpsimd-queue issues cost ~1us of Pool.
        # DMA issue overhead is ~0.8us per transfer regardless of size, so
        # startup uses FEW, BIG transfers in dependency order: x0 per-q
        # (gates stats), mh/ml whole (gate w(0)); the tiny GN constants ride
        # the scalar queue in parallel. proj weights deferred to iter 0 end.
        xtp = ctx.enter_context(tc.tile_pool(name="xtp", bufs=4))
        xts = {}
        xts[0] = xtp.tile([P, CT, HW], bf16, tag="xt", name="xt0")
        xv0 = x_d[0].rearrange("(q p) f -> p q f", p=P)
        for q in range(CT):
            nc.sync.dma_start(xts[0][:, q], xv0[:, q])
        selg_sb = singles.tile([P, 8], f32)
        nc.scalar.dma_start(selg_sb, sg_d)
        gw_sb = singles.tile([P, CT], f32)
        nc.scalar.dma_start(gw_sb, gw_d)
        gb_sb = singles.tile([P, CT], f32)
        nc.scalar.dma_start(gb_sb, gb_d)
        vu_sb = singles.tile([P, CT, 1], bf16)
        nc.scalar.dma_start(vu_sb, vu_d)
        ss_sb = singles.tile([P, IJT, G], ss_dt)
        nc.scalar.dma_start(ss_sb, ss_d)
        pb_sb = singles.tile([P, CT], f32)
        nc.scalar.dma_start(pb_sb, pb_d)
        mh_sb = singles.tile([P, CT, C], f8)
        nc.sync.dma_start(mh_sb, mh_d)
        ml_sb = singles.tile([P, CT, C], f8)
        nc.sync.dma_start(ml_sb, ml_d)
        ident_sb = singles.tile([P, P], bf16)
        nc.sync.dma_start(ident_sb, id_d)

        small = ctx.enter_context(tc.tile_pool(name="small", bufs=1))
        stp = ctx.enter_context(tc.tile_pool(name="stp", bufs=4))
        epsb = singles.tile([P, 1], f32)
        nc.vector.memset(epsb, EPS)
        # warm the Exp ACT table set while the first DMAs run
        actwarm = singles.tile([P, 1], f32)
        nc.scalar.activation(out=actwarm, in_=epsb, func=AF.Exp)
        magic = singles.tile([8, CT, 1], dt.int32)
        nc.vector.memset(magic, 0x5F3759DF)
        shiftb = singles.tile([P, 1], f32)
        nc.vector.memset(shiftb, -SHIFT)
        gst = singles.tile([8, BS * CT, 2], f32)
        scv = singles.tile([P, BS * CT], f32)
        tcv = singles.tile([P, BS * CT], f32)

        def stats_q_closures(s):
            """Per-q channel bn_stats -> PE select-matmul group partial;
            interleaved into the previous sample's scores stream so the tiny
            pg matmuls never stall the PE behind the DVE stats chain."""
            xt = xts[s]

            def one(q):
                stq = stp.tile([P, 2, 6], f32, tag="stq")
                for sub in range(2):
                    nc.vector.bn_stats(
                        out=stq[:, sub, :], in_=xt[:, q, sub * 512 : (sub + 1) * 512]
                    )
                mvq = stp.tile([P, 2], f32, tag="mvq")
                nc.vector.bn_aggr(out=mvq, in_=stq)
                exq = stp.tile([P, 2], f32, tag="exq")
                nc.vector.tensor_copy(out=exq[:, 0:1], in_=mvq[:, 0:1])
                nc.vector.tensor_scalar(
                    exq[:, 1:2], mvq[:, 0:1], mvq[:, 0:1], mvq[:, 1:2],
                    op0=ALU.mult, op1=ALU.add,
                )
                pg = pgp.tile([8, 2], f32, tag="pg")
                nc.tensor.matmul(pg, lhsT=selg_sb, rhs=exq, start=True, stop=True)
                nc.vector.tensor_copy(out=gst[0:8, s * CT + q, :], in_=pg)

            return [lambda q=q: one(q) for q in range(CT)]

        def emit_stats_combine(s):
            gm = gst[0:8, s * CT : (s + 1) * CT, 0:1]
            gx2 = gst[0:8, s * CT : (s + 1) * CT, 1:2]
            # group combine + Newton rsqrt on Pool: DVE must stay clear for
            # the previous sample's softmax chain
            veng = nc.gpsimd
            gv = stp.tile([8, CT, 1], f32, tag="gv")
            veng.tensor_tensor(gv, gm, gm, ALU.mult)
            veng.tensor_tensor(gv, gx2, gv, ALU.subtract)
            veng.tensor_scalar(gv, gv, EPS, None, op0=ALU.add)
            i32 = dt.int32
            yb = stp.tile([8, CT, 1], f32, tag="yb")
            nc.vector.tensor_scalar(
                yb.bitcast(i32), gv.bitcast(i32), 1, None,
                op0=ALU.arith_shift_right,
            )
            nc.vector.tensor_tensor(
                yb.bitcast(i32), magic, yb.bitcast(i32), ALU.subtract
            )
            hh = stp.tile([8, CT, 1], f32, tag="hh")
            veng.tensor_scalar(hh, gv, 0.5, None, op0=ALU.mult)
            ttn = stp.tile([8, CT, 1], f32, tag="ttn")
            for _ in range(2):
                veng.tensor_tensor(ttn, yb, yb, ALU.mult)
                veng.tensor_tensor(ttn, hh, ttn, ALU.mult)
                veng.tensor_scalar(
                    ttn, ttn, -1.0, 1.5, op0=ALU.mult, op1=ALU.add
                )
                veng.tensor_tensor(yb, yb, ttn, ALU.mult)
            gv = yb
            rstdb = stp.tile([P, CT], f32, tag="rstdb")
            with tc.high_priority(offset=1 << 20):
                nc.sync.dma_start(
                    rstdb.opt(keep_dims={0}), pbc(gv[0:8, :, 0], 16)
                )
            gmt = stp.tile([8, CT, 1], f32, tag="gmt")
            veng.tensor_copy(out=gmt, in_=gm)
            gmb = stp.tile([P, CT], f32, tag="gmb")
            with tc.high_priority(offset=1 << 20):
                nc.sync.dma_start(
                    gmb.opt(keep_dims={0}), pbc(gmt[0:8, :, 0], 16)
                )
            cs = scv[:, s * CT : (s + 1) * CT]
            veng.tensor_tensor(cs, gw_sb, rstdb, ALU.mult)
            tmpb = stp.tile([P, CT], f32, tag="tmpb")
            veng.tensor_tensor(tmpb, gmb, cs, ALU.mult)
            veng.tensor_tensor(
                tcv[:, s * CT : (s + 1) * CT], gb_sb, tmpb, ALU.subtract
            )

        # PE p-state warmup: dummy matmuls on a zeroed tile keep the PE busy
        # from t~0 so the pipeline is ramped (2.4GHz needs ~3us of continuous
        # work) when the first real matmuls arrive at ~6us
        warm = singles.tile([P, NF], bf16)
        nc.vector.memset(warm.bitcast(dt.uint16), 0)
        pwarm = pgp.tile([2, NF], f32, tag="pg")
        for _ in range(10):
            nc.tensor.matmul(pwarm, lhsT=warm[:, 0:2], rhs=warm,
                             start=True, stop=True)

        def emit_stats(s):
            for f in stats_q_closures(s):
                f()
            emit_stats_combine(s)

        emit_stats(0)

        ptmp = ctx.enter_context(tc.tile_pool(name="ptmp", bufs=4))
        bigs = ctx.enter_context(tc.tile_pool(name="bigs", bufs=1))
        rbp = ctx.enter_context(tc.tile_pool(name="rbp", bufs=8))

        # ---- per-sample attention ----
        nts, nhs, nls = {}, {}, {}

        def emit_normalize_t(s):
            # t = GN(x) in bf16 and nh = fp8(t), on Pool (steady state);
            # sample 0 alternates engines per 512-wide half on the startup
            # critical path. The nl = fp8(t - nh) DVE pass is emitted
            # separately (emit_normalize_lo) so it lands on DVE's queue after
            # the previous sample's softmax chain.
            nt = bigs.tile([P, CT, HW], bf16, tag="nt", bufs=2, name=f"nt{s}")
            nh = bigs.tile([P, CT, HW], f8, tag="nh", bufs=2, name=f"nh{s}")
            nl = bigs.tile([P, CT, HW], f8, tag="nl", bufs=2, name=f"nl{s}")
            nts[s], nhs[s], nls[s] = nt, nh, nl
            if s == 0:
                for h2 in range(2 * CT):
                    q, sub = divmod(h2, 2)
                    e0 = nc.vector if h2 % 2 == 0 else nc.gpsimd
                    e1 = nc.gpsimd if h2 % 2 == 0 else nc.vector
                    sl = slice(sub * NF, (sub + 1) * NF)
                    e0.tensor_scalar(
                        nt[:, q, sl], xts[s][:, q, sl],
                        scv[:, s * CT + q : s * CT + q + 1],
                        tcv[:, s * CT + q : s * CT + q + 1],
                        op0=ALU.mult, op1=ALU.add,
                    )
                    e1.tensor_copy(out=nh[:, q, sl], in_=nt[:, q, sl])
                    e0.tensor_tensor(nl[:, q, sl], nt[:, q, sl], nh[:, q, sl],
                                     ALU.subtract)
                return
            for q in range(CT):
                # t on ACT: activation natively does x*scale+bias with
                # per-partition APs at ~2x Pool's tensor_scalar speed, so the
                # s+1 normalize chain clears well before w(s+1) needs it
                nc.scalar.activation(
                    out=nt[:, q], in_=xts[s][:, q], func=AF.Identity,
                    scale=scv[:, s * CT + q : s * CT + q + 1],
                    bias=tcv[:, s * CT + q : s * CT + q + 1],
                )
                nc.gpsimd.tensor_copy(out=nh[:, q], in_=nt[:, q])

        def emit_normalize_lo(s):
            if s == 0:
                return
            nt, nh, nl = nts[s], nhs[s], nls[s]
            for q in range(CT):
                nc.vector.tensor_tensor(nl[:, q], nt[:, q], nh[:, q],
                                        ALU.subtract)

        emit_normalize_t(0)
        pth_sb = singles.tile([P, CT, C], f8)
        ptl_sb = singles.tile([P, CT, C], f8)
        whs, wls, vts, ess, e8s, uss, hhs, hls = {}, {}, {}, {}, {}, {}, {}, {}

        def dr6(ps_n, lhi, llo, rhi, rlo, ocols, ncols):
            # hi*hi + hi*lo + lo*hi via 6 DoubleRow matmuls (2 k-tile pairs
            # each) accumulating into one psum
            idx = 0
            for L, R in ((lhi, rhi), (lhi, rlo), (llo, rhi)):
                for kp in range(CT // 2):
                    nc.tensor.matmul(
                        ps_n,
                        lhsT=L[:, 2 * kp : 2 * kp + 2, ocols],
                        rhs=R[:, 2 * kp : 2 * kp + 2, ncols],
                        start=(idx == 0), stop=(idx == 5),
                        perf_mode=DR,
                    )
                    idx += 1

        def w_groups(s):
            # w = (Wq'^T Wk') n at xMTS: split-fp8 DoubleRow; psum lands at
            # the fp8 target scale so hi = ACT copy, lo = DVE psum subtract
            nh, nl = nhs[s], nls[s]
            wh = bigs.tile([P, CT, HW], f8, tag="wh", bufs=2, name=f"wh{s}")
            wl = bigs.tile([P, CT, HW], f8, tag="wl", bufs=2, name=f"wl{s}")
            whs[s], wls[s] = wh, wl

            def w_one(ot):
                ps = pmm.tile([P, NH, NF], f32, tag="mm")
                for n in range(NH):
                    dr6(ps[:, n], mh_sb, ml_sb, nh, nl,
                        slice(ot * P, (ot + 1) * P),
                        slice(n * NF, (n + 1) * NF))
                nc.scalar.activation(out=wh[:, ot], in_=ps, func=AF.Copy)
                # (Pool cannot read PSUM, so the lo-subtract must be on DVE)
                nc.vector.tensor_tensor(wl[:, ot], ps, wh[:, ot], ALU.subtract)

            return [lambda ot=ot: w_one(ot) for ot in range(CT)]

        def emit_u(s):
            # u[ij] = bq'^T k'[:,ij] - SHIFT, the per-partition exp bias.
            # All IJT columns accumulate in one psum bank (independent
            # accumulation groups per f32 column) -> a single ACT copy.
            usb = small.tile([P, IJT], f32, tag="u", bufs=2, name=f"u{s}")
            uss[s] = usb
            nt = nts[s]
            pu = pup.tile([P, IJT], f32, tag="pu")
            for t in range(IJT):
                for k in range(CT):
                    nc.tensor.matmul(
                        pu[:, t : t + 1],
                        lhsT=nt[:, k, t * P : (t + 1) * P],
                        rhs=vu_sb[:, k],
                        start=(k == 0), stop=(k == CT - 1),
                    )
            nc.scalar.activation(
                out=usb, in_=pu, func=AF.Identity, bias=shiftb,
            )

        def vt_groups(s):
            # n^T[ij, c] via PE transpose of the bf16 t
            nt = nts[s]
            vtsb = bigs.tile([P, IJT, C], bf16, tag="vt", bufs=2, name=f"ntt{s}")
            vts[s] = vtsb

            def vt_one(t2):
                ps = pmm.tile([P, 2, NF], bf16, tag="mm")
                for i2 in range(2):
                    t = 2 * t2 + i2
                    for k in range(CT):
                        nc.tensor.transpose(
                            ps[:, i2, k * P : (k + 1) * P],
                            nt[:, k, t * P : (t + 1) * P],
                            ident_sb,
                        )
                nc.scalar.activation(
                    out=vtsb[:, 2 * t2 : 2 * t2 + 2], in_=ps, func=AF.Identity
                )

            return [lambda t2=t2: vt_one(t2) for t2 in range(IJT // 2)]

        pds = {}

        def _emit_d(s, t, pd0, pd1):
            # both n-halves' D accumulation steps for key tile t
            for n, pd in ((0, pd0), (1, pd1)):
                if fp8_d:
                    if t % 2 == 1:
                        nc.tensor.matmul(
                            pd,
                            lhsT=ss_sb[:, t - 1 : t + 1, :],
                            rhs=e8s[s][:, t - 1 : t + 1,
                                       n * NF : (n + 1) * NF],
                            start=(t == 1), stop=(t == IJT - 1),
                            perf_mode=DR,
                        )
                else:
                    nc.tensor.matmul(
                        pd,
                        lhsT=ss_sb[:, t, :],
                        rhs=ess[s][:, t, n * NF : (n + 1) * NF],
                        start=(t == 0), stop=(t == IJT - 1),
                    )

        def emit_scores_exp(s, qfill=()):
            # S^T[ij, hw] at xMTS via split-fp8 DoubleRow; E = exp(S + u -
            # SHIFT) in bf16. The D-reduction matmuls for tile t-1 are
            # interleaved after the score matmuls of tile t: D(t-1) waits on
            # exp(t-1), which by then has completed, so the whole D phase is
            # absorbed into the scores stream. `qfill` (next sample's per-q
            # stats) rides the same stream after the odd tiles.
            qfill = list(qfill)
            wh, wl, nh, nl, usb = whs[s], wls[s], nhs[s], nls[s], uss[s]
            esb = bigs.tile([P, IJT, HW], bf16, tag="E", bufs=2, name=f"E{s}")
            ess[s] = esb
            if fp8_d:
                e8 = bigs.tile([P, IJT, HW], f8, tag="E8", bufs=2, name=f"E8{s}")
                e8s[s] = e8
            pd0 = pdp.tile([G, NF], f32, tag="pd")
            pd1 = pdp.tile([G, NF], f32, tag="pd")
            pds[s] = (pd0, pd1)
            for t in range(IJT):
                ps = pmm.tile([P, NH, NF], f32, tag="mm")
                for n in range(NH):
                    dr6(ps[:, n], wh, wl, nh, nl,
                        slice(t * P, (t + 1) * P),
                        slice(n * NF, (n + 1) * NF))
                nc.scalar.activation(
                    out=esb[:, t], in_=ps, func=AF.Exp,
                    bias=usb[:, t : t + 1], scale=1.0 / MTS,
                )
                if fp8_d:
                    nc.gpsimd.tensor_copy(out=e8s[s][:, t], in_=esb[:, t])
                if t > 0:
                    _emit_d(s, t - 1, pd0, pd1)
                if t % 2 == 1 and (t - 1) // 2 < len(qfill):
                    qfill[(t - 1) // 2]()
            _emit_d(s, IJT - 1, pd0, pd1)

        def emit_softmax_tail(s):
            # R = AS/D via DVE fast-recip + Newton into the bf16 broadcast
            # source, A = E * broadcast(R) on DVE in place. No PE work: the
            # PE covers this window with proj(s-1) + vT(s), emitted after.
            from concourse.dve_ops import RECIPROCAL_APPROX_NR

            esb = ess[s]
            pd0, pd1 = pds[s]
            rsc = small.tile([G, HW], f32, tag="rsc")
            rrb = small.tile([G, HW], bf16, tag="rrb")
            for n, pd in ((0, pd0), (1, pd1)):
                nc.vector.reciprocal_approx_fast(
                    out=rsc[:, n * NF : (n + 1) * NF], in_=pd
                )
                nc.vector._custom_dve(
                    RECIPROCAL_APPROX_NR,
                    out=rrb[:, n * NF : (n + 1) * NF],
                    in0=pd,
                    in1=rsc[:, n * NF : (n + 1) * NF],
                    s0=2.0,
                )
            for t in range(IJT):
                rbt = rbp.tile([P, HW], bf16, tag="rb")
                (nc.sync, nc.gpsimd, nc.scalar)[t % 3].dma_start(
                    rbt, pbc(rrb[4 * t : 4 * t + 4, :], 32)
                )
                nc.vector.tensor_tensor(esb[:, t], esb[:, t], rbt, ALU.mult)

        def emit_h(s):
            # h[c, hw] at xAS = sum_ij v^T[ij,c] A^T[ij,hw] in bf16, split to
            # fp8 hi/lo from psum for the split-fp8 proj
            vtsb, esb = vts[s], ess[s]
            hh = bigs.tile([P, CT, HW], f8, tag="hh", bufs=2, name=f"hh{s}")
            hl = bigs.tile([P, CT, HW], f8, tag="hl", bufs=2, name=f"hl{s}")
            hhs[s], hls[s] = hh, hl
            for ct in range(CT):
                ps = pmm.tile([P, NH, NF], f32, tag="mm")
                for n in range(NH):
                    for t in range(IJT):
                        nc.tensor.matmul(
                            ps[:, n],
                            lhsT=vtsb[:, t, ct * P : (ct + 1) * P],
                            rhs=esb[:, t, n * NF : (n + 1) * NF],
                            start=(t == 0), stop=(t == IJT - 1),
                        )
                nc.scalar.activation(out=hh[:, ct], in_=ps, func=AF.Copy)
                nc.vector.tensor_tensor(hl[:, ct], ps, hh[:, ct], ALU.subtract)

        def proj_groups(s):
            # split-fp8 proj + bias + residual, then store
            hh, hl, xt = hhs[s], hls[s], xts[s]
            ov = out_d[s].rearrange("(q p) f -> p q f", p=P)

            def p_one(ot):
                ps = pmm.tile([P, NH, NF], f32, tag="mm")
                for n in range(NH):
                    dr6(ps[:, n], pth_sb, ptl_sb, hh, hl,
                        slice(ot * P, (ot + 1) * P),
                        slice(n * NF, (n + 1) * NF))
                tmp = ptmp.tile([P, HW], f32, tag="pt")
                nc.scalar.activation(
                    out=tmp, in_=ps, func=AF.Identity,
                    bias=pb_sb[:, ot : ot + 1], scale=1.0 / (PTS * AS),
                )
                nc.vector.tensor_tensor(tmp, tmp, xt[:, ot], ALU.add)
                nc.sync.dma_start(ov[:, ot], tmp)

            return [lambda ot=ot: p_one(ot) for ot in range(CT)]

        def emit_tail(s):
            # final sample: h and proj at n-half granularity so the first
            # half's proj/residual/stores overlap the second half's h matmuls
            vtsb, esb, xt = vts[s], ess[s], xts[s]
            hh = bigs.tile([P, CT, HW], f8, tag="hh", bufs=2, name=f"hh{s}")
            hl = bigs.tile([P, CT, HW], f8, tag="hl", bufs=2, name=f"hl{s}")
            ov = out_d[s].rearrange("(q p) f -> p q f", p=P)
            for n in range(NH):
                sl = slice(n * NF, (n + 1) * NF)
                for ct in range(CT):
                    ps = pmm.tile([P, NF], f32, tag="mm")
                    for t in range(IJT):
                        nc.tensor.matmul(
                            ps,
                            lhsT=vtsb[:, t, ct * P : (ct + 1) * P],
                            rhs=esb[:, t, sl],
                            start=(t == 0), stop=(t == IJT - 1),
                        )
                    nc.scalar.activation(out=hh[:, ct, sl], in_=ps,
                                         func=AF.Copy)
                    nc.vector.tensor_tensor(hl[:, ct, sl], ps, hh[:, ct, sl],
                                            ALU.subtract)
                for ot in range(CT):
                    ps = pmm.tile([P, NF], f32, tag="mm")
                    dr6(ps, pth_sb, ptl_sb, hh, hl,
                        slice(ot * P, (ot + 1) * P), sl)
                    tmp = ptmp.tile([P, NF], f32, tag="pt")
                    nc.scalar.activation(
                        out=tmp, in_=ps, func=AF.Identity,
                        bias=pb_sb[:, ot : ot + 1], scale=1.0 / (PTS * AS),
                    )
                    nc.vector.tensor_tensor(tmp, tmp, xt[:, ot, sl], ALU.add)
                    (nc.sync if ot % 2 == 0 else nc.scalar).dma_start(
                        ov[:, ot, sl], tmp
                    )

        # software pipeline: scores/exp(s) -> stats(s+1) -> softmax(s) with
        # proj(s-1) interleaved between D matmuls -> h(s) -> normalize(s+1)
        # and w/u/vT(s+1) (emitted after h so the PE does not stall on the
        # s+1 normalize chain, which runs behind the E8 casts on Pool)
        for f in w_groups(0):
            f()
        emit_u(0)
        for s in range(BS):
            emit_scores_exp(s)
            if s + 1 < BS:
                xts[s + 1] = xtp.tile([P, CT, HW], bf16, tag="xt",
                                      name=f"xt{s + 1}")
                xvn = x_d[s + 1].rearrange("(q p) f -> p q f", p=P)
                for q in range(CT):
                    nc.sync.dma_start(xts[s + 1][:, q], xvn[:, q])
                emit_stats(s + 1)
                emit_normalize_t(s + 1)
            emit_softmax_tail(s)
            if s >= 1:
                for f in proj_groups(s - 1):
                    f()
            for f in vt_groups(s):
                f()
            if s + 1 < BS:
                emit_normalize_lo(s + 1)
                emit_h(s)
                for f in w_groups(s + 1):
                    f()
                emit_u(s + 1)
                if s == 0:
                    # proj weights: needed only by proj(0), deep in iter 1
                    nc.scalar.dma_start(pth_sb, ph_d)
                    nc.scalar.dma_start(ptl_sb, pl_d)
            else:
                emit_tail(s)

    nc.compile()
    return nc


def _prep_inputs(x, gn_w, gn_b, qkv_w, qkv_b, proj_w, proj_b):
    import ml_dtypes

    f8 = ml_dtypes.float8_e4m3
    bfnp = ml_dtypes.bfloat16

    x = np.asarray(x, dtype=np.float32)
    gn_w = np.asarray(gn_w, dtype=np.float32)
    gn_b = np.asarray(gn_b, dtype=np.float32)
    qkv_w = np.asarray(qkv_w, dtype=np.float32)
    qkv_b = np.asarray(qkv_b, dtype=np.float32)
    proj_w = np.asarray(proj_w, dtype=np.float32)
    proj_b = np.asarray(proj_b, dtype=np.float32)

    s4 = np.float32(float(C) ** -0.25)
    Wq = (qkv_w[:C] * s4).astype(np.float64)
    Wk = (qkv_w[C : 2 * C] * s4).astype(np.float64)
    bq = (qkv_b[:C] * s4).astype(np.float64)
    # Gram fold: S = n^T (Wq^T Wk) n + (Wk^T bq).n_ij, at xMTS for fp8
    mt = (Wk.T @ Wq).astype(np.float32) * np.float32(MTS)   # [C, C] c_in x o
    mh = mt.astype(f8)
    ml_ = (mt - mh.astype(np.float32)).astype(f8)

    def karr(a):
        # [C, C] (c_in, o) -> [P, CT, C] with c_in = k*P + p
        return np.ascontiguousarray(a.reshape(CT, P, C).transpose(1, 0, 2))

    vu = np.ascontiguousarray(
        (Wk.T @ bq).astype(np.float32).reshape(CT, P).T[:, :, None]
    ).astype(bfnp)                                          # [P, CT, 1]
    Wv = qkv_w[2 * C :].astype(np.float64)
    pt = ((proj_w.astype(np.float64) @ Wv).T.astype(np.float32)
          * np.float32(PTS))                                # [C, C] c_in x o
    pth = pt.astype(f8)
    ptl = (pt - pth.astype(np.float32)).astype(f8)
    ident = np.eye(P, dtype=np.float32).astype(bfnp)
    vb = qkv_b[2 * C :]
    pb = np.ascontiguousarray(
        (proj_b + np.float32(H) * (proj_w @ vb)).reshape(CT, P).T
    )                                                       # [P, CT]
    gw = np.ascontiguousarray(gn_w.reshape(CT, P).T)
    gb = np.ascontiguousarray(gn_b.reshape(CT, P).T)
    selg = np.zeros((P, 8), dtype=np.float32)
    selg[np.arange(P), np.arange(P) // 16] = 1.0 / 16.0
    ss = np.zeros((P, IJT, G), dtype=np.float32)
    for t in range(IJT):
        for p in range(P):
            ss[p, t, 4 * t + p // G] = 1.0 / AS
    ss = ss.astype(f8 if D_MODE == "fp8" else bfnp)
    shared = {
        "mh": karr(mh), "ml": karr(ml_), "vu": vu,
        "pth": karr(pth), "ptl": karr(ptl), "pb": pb,
        "ident": ident, "gw": gw, "gb": gb, "ssum": ss, "selg": selg,
    }
    in_maps = []
    for c in range(NCORES):
        m = dict(shared)
        m["x"] = np.ascontiguousarray(
            x[c * BS : (c + 1) * BS].reshape(BS, C, HW)
        ).astype(bfnp)
        in_maps.append(m)
    return in_maps


def run(inputs: dict, trace: bool = False, n_cores: int = NCORES):
    """Build (cached), run on hardware, return BassKernelResults."""
    from concourse.bass_utils import run_bass_kernel_spmd

    key = MM_MODE
    if key not in _cache:
        _cache[key] = _build(D_MODE)
    nc = _cache[key]
    in_maps = _prep_inputs(**inputs)[:n_cores]
    res = run_bass_kernel_spmd(nc, in_maps, list(range(n_cores)), trace=trace)
    return res


def kernel(x, gn_w, gn_b, qkv_w, qkv_b, proj_w, proj_b) -> np.ndarray:
    res = run(dict(x=x, gn_w=gn_w, gn_b=gn_b, qkv_w=qkv_w, qkv_b=qkv_b,
                   proj_w=proj_w, proj_b=proj_b))
    out = np.concatenate(
        [res.results[c]["out"].reshape(BS, C, H, W) for c in range(NCORES)], axis=0
    )
    return out


# revision 66
# speedup vs baseline: 1.2873x; 1.0919x over previous
"""AttentionBlock (GroupNorm -> 1x1 qkv conv -> spatial attention with
softmax over the last width axis -> 1x1 proj conv -> residual) on 8
Trainium2 NeuronCores, data-parallel over the batch.

Self-contained: hardcodes shapes B,C,H,W = 32,512,32,32 and the
8-core batch sharding. Host-side preprocessing folds the C**-0.25
attention scale into the q/k weight rows, transposes the 1x1-conv
weights, and folds the v bias into the proj bias (sum_ij softmax_j(S)
== H exactly). On-device, per sample: GroupNorm stats via channel-wise
bn_stats + a tiny PE select-matmul for the 16-channel group combine
(rsqrt = DVE quake-seed + 2 Newton steps, so only the Exp ACT table is
ever loaded); qkv/scores/attn@v/proj as PE matmuls with the softmax
done in "scores transposed" orientation (ij on partitions, v computed
transposed directly): softmax-over-j denominators are a 128-wide
select-matrix PE matmul accumulated in psum, 1/D is a custom DVE
approx op reading psum directly and writing the bf16 broadcast source,
and the i->32-j-partition broadcast is a replicating DMA (issues
rotated over the SP/ACT/Pool queues) with the A-multiply on DVE. The
scores use the host-folded Gram form S = n^T (Wq'^T Wk') n (+ the
j-dependent bias via the exp's per-partition bias slot; hw-dependent
terms are softmax-invariant and dropped), which removes the separate
q/k projections entirely.

Precision (ATTN_MM_MODE env): "f32r" (default) runs the GN/scores/proj
path in float32r (single-pass PE fp32, 1 cycle/row vs 4 for f32; the
small/odd-shaped matmuls are padded or kept f32 to satisfy the
s3d3_mm_fp32r ISA restrictions) and the attention interior (E, v^T,
softmax select) in bf16, and x itself ships host-converted to bf16
(frees 32KB/partition of SBUF, halves the input wire time; the +x
residual is re-widened into the f32 proj-bias tile before the store)
-- rel err ~3e-3 vs the 2e-2 gate, ~228us/core
under the TimelineSim cost model (~3.7x over the all-f32 baseline at
~871us; "mix" keeps the outer path in true f32 for debugging at
~563us). Scheduling: per-sample phases are software-pipelined (next
sample's GN stats/normalize on Pool, its w/u/v^T matmuls and the
previous sample's deferred proj fill this sample's softmax window);
psum is split 2x2-bank pmm pairs (1024-wide ACT psum->sbuf ops) +
3+1-bank pools for the D/u accumulators, and the broadcast tile pool
is 8-deep so the DMA stream runs ahead of the multiplies, which under
the greedy tile scheduler keeps the PE ~84% busy (~191us of 228us;
startup GN chain and the last sample's softmax tail account for the
rest). The broadcast-DMA queue rotation (sync/gpsimd/scalar, in that
phase) is load-bearing: other phases cost 1-20us.
"""

import os
from contextlib import ExitStack

import numpy as np

B, C, H, W = 32, 512, 32, 32
HW = H * W            # 1024
G = 32                # groupnorm groups
GS = C // G           # 16 channels per group
NCORES = 8
BS = B // NCORES      # 4 samples per core
EPS = 1e-5
P = 128
CT = C // P           # 4 channel tiles
IJT = HW // P         # 8 key-pixel tiles
NF = 512              # matmul moving free dim
NH = HW // NF         # 2

# "f32r" (default): single-pass fp32 outer path + bf16 attention interior
# "mix"           : true-f32 outer path (4 cyc/row) + bf16 interior (debug)
MM_MODE = os.environ.get("ATTN_MM_MODE", "f32r")

_cache: dict = {}


def _build(mm_mode: str):
    import concourse.bass as bass
    import concourse.tile as tile
    from concourse import bacc, mybir

    dt = mybir.dt
    AF = mybir.ActivationFunctionType
    ALU = mybir.AluOpType
    f32 = dt.float32
    f32r = dt.float32r
    bf16 = dt.bfloat16
    # matmul-operand dtypes: mdt covers the GN/scores/proj path (f32r =
    # single-pass PE fp32, 4x faster than f32); mdt_att covers the
    # attention interior (E, v^T, softmax select/broadcast) where bf16 is
    # ample (values are softmax weights in [0,1] and normalized v), buys
    # 2x SBUF and the DVE 2-byte fast paths, and keeps 1 PE cycle/row.
    if mm_mode == "f32":
        # the all-f32 reference build no longer fits SBUF under the
        # pipelined schedule (nt x3 / vt x2 buffering); "mix" keeps the
        # GN/scores/proj path in true f32 for debugging
        raise ValueError(
            "ATTN_MM_MODE=f32 is no longer supported; use 'mix' "
            "(f32 outer path, bf16 attention interior) or the default 'f32r'"
        )
    elif mm_mode == "f32r":
        mdt, mdt_att = f32r, bf16
    else:  # mix
        mdt, mdt_att = f32, bf16

    nc = bacc.Bacc("TRN2", target_bir_lowering=False, debug=False,
                   dynamic_dma_scratch_size=8192)

    x_d = nc.dram_tensor("x", [BS, C, HW], mdt_att, kind="ExternalInput").ap()
    mt_d = nc.dram_tensor("mt", [C, C], mdt, kind="ExternalInput").ap()
    id_d = nc.dram_tensor("ident", [P, P], mdt, kind="ExternalInput").ap()
    # vu duplicated to 2 columns: fp32r matmuls need an even moving-dim
    # element count (s3d3_mm_fp32r_restrictions)
    vu_d = nc.dram_tensor("vu", [P, CT, 2], mdt, kind="ExternalInput").ap()
    pt_d = nc.dram_tensor("pt", [C, C], mdt, kind="ExternalInput").ap()
    pb_d = nc.dram_tensor("pb", [P, CT], f32, kind="ExternalInput").ap()
    gw_d = nc.dram_tensor("gw", [P, CT], f32, kind="ExternalInput").ap()
    gb_d = nc.dram_tensor("gb", [P, CT], f32, kind="ExternalInput").ap()
    # selg stays plain f32: its [8,2] matmul output violates the fp32r
    # col_grp==0xf restriction; it is tiny so f32 (4 cyc/row) is free.
    # ssum's free dim is padded 32 -> 128 output rows (rows 32..127 all
    # zero) for the same col_grp restriction; only psum rows 0..31 are
    # read back.
    sg_d = nc.dram_tensor("selg", [P, 8], f32, kind="ExternalInput").ap()
    ss_d = nc.dram_tensor("ssum", [P, IJT * P], mdt_att, kind="ExternalInput").ap()
    out_d = nc.dram_tensor("out", [BS, C, HW], f32, kind="ExternalOutput").ap()

    with tile.TileContext(nc) as tc, ExitStack() as ctx:
        singles = ctx.enter_context(tc.tile_pool(name="singles", bufs=1))
        # pmm tiles are 2-bank [P, NH, NF] pairs so the ACT psum->sbuf ops
        # run 1024-wide (halves ACT op count + per-op init overhead)
        pmm = ctx.enter_context(tc.tile_pool(name="pmm", bufs=2, space="PSUM"))
        pdp = ctx.enter_context(tc.tile_pool(name="pdp", bufs=3, space="PSUM"))
        pup = ctx.enter_context(tc.tile_pool(name="pup", bufs=1, space="PSUM"))

        def pbc(base, rep):
            # partition-broadcast source AP: replicate each source partition
            # `rep` times (destination iterates partitions major)
            base = base.opt(keep_dims={0})
            ap = [d for d in base.ap[1:] if d[1] > 1] or [[1, 1]]
            return bass.AP(
                tensor=base.tensor, offset=base.offset,
                ap=[base.ap[0], [0, rep], *ap],
            )

        # startup DMA: the transfers serialize through one DMA-engines
        # resource with ~0.8us issue overhead each, so use FEW, BIG
        # transfers in dependency order. sync: x0 per-q (gates stats) then
        # the qkv Gram matrix + ident (gate w(0)); the small constants ride
        # the scalar queue in parallel. The proj weight stays deferred.
        xtp = ctx.enter_context(
            tc.tile_pool(name="xtp", bufs=4 if mdt_att is not f32 else 3)
        )
        xts = {}
        xts[0] = xtp.tile([P, CT, HW], mdt_att, tag="xt", name="xt0")
        xv0 = x_d[0].rearrange("(q p) f -> p q f", p=P)
        for q in range(CT):
            nc.sync.dma_start(xts[0][:, q], xv0[:, q])
        gw_sb = singles.tile([P, CT], f32)
        nc.scalar.dma_start(gw_sb, gw_d)
        gb_sb = singles.tile([P, CT], f32)
        nc.scalar.dma_start(gb_sb, gb_d)
        selg_sb = singles.tile([P, 8], f32)
        nc.scalar.dma_start(selg_sb, sg_d)
        vu_sb = singles.tile([P, CT, 2], mdt)
        nc.scalar.dma_start(vu_sb, vu_d)
        pb_sb = singles.tile([P, CT], f32)
        nc.scalar.dma_start(pb_sb, pb_d)
        ss_sb = singles.tile([P, IJT, P], mdt_att)
        nc.scalar.dma_start(ss_sb, ss_d.rearrange("p (t g) -> p t g", t=IJT))

        # mt = Wk'^T Wq' (the scores Gram matrix, host-folded) is needed
        # first; ident feeds the PE transpose mode that produces n^T
        mt_sb = singles.tile([P, CT, C], mdt)
        mtv = mt_d.rearrange("(k p) o -> p k o", p=P)
        for half in range(2):
            nc.sync.dma_start(
                mt_sb[:, :, half * 256 : (half + 1) * 256],
                mtv[:, :, half * 256 : (half + 1) * 256],
            )
        ident_sb = singles.tile([P, P], mdt)
        nc.sync.dma_start(ident_sb, id_d)

        small = ctx.enter_context(tc.tile_pool(name="small", bufs=1))
        stp = ctx.enter_context(tc.tile_pool(name="stp", bufs=4))
        epsb = singles.tile([P, 1], f32)
        nc.vector.memset(epsb, EPS)
        # warm the Exp ACT table set while the first DMAs run (the only
        # table-based ACT function this kernel uses)
        actwarm = singles.tile([P, 1], f32)
        nc.scalar.activation(out=actwarm, in_=epsb, func=AF.Exp)
        magic = singles.tile([8, CT, 1], dt.int32)
        nc.vector.memset(magic, 0x5F3759DF)
        # PE p-state warmup: dummy matmuls keep the PE busy from t~0 so the
        # pipeline is ramped (2.4GHz needs ~3us of continuous work) when the
        # first real matmuls arrive at ~6us
        warm = singles.tile([P, NF], bf16)
        nc.vector.memset(warm.bitcast(dt.uint16), 0)
        nwarm = int(os.environ.get("ATTN_WARM", "0"))
        if nwarm:
            pwarm = pdp.tile([2, NF], f32, tag="pd")
            for _ in range(nwarm):
                nc.tensor.matmul(pwarm, lhsT=warm[:, 0:2], rhs=warm,
                                 start=True, stop=True)
        # per-(sample,group) stats: [8 group-in-qtile, (s,q), (mean, E[x^2])]
        gst = singles.tile([8, BS * CT, 2], f32)
        scv = singles.tile([P, BS * CT], f32)
        tcv = singles.tile([P, BS * CT], f32)

        def emit_stats(s):
            """Channel bn_stats on xt(s) -> group combine on PE -> per-channel
            GN scale/offset columns scv/tcv[:, s*CT..]."""
            xt = xts[s]
            for q in range(CT):
                stq = stp.tile([P, 2, 6], f32, tag="stq")
                for sub in range(2):
                    nc.vector.bn_stats(
                        out=stq[:, sub, :], in_=xt[:, q, sub * 512 : (sub + 1) * 512]
                    )
                mvq = stp.tile([P, 2], f32, tag="mvq")
                nc.vector.bn_aggr(out=mvq, in_=stq)
                exq = stp.tile([P, 2], f32, tag="exq")
                nc.vector.tensor_copy(out=exq[:, 0:1], in_=mvq[:, 0:1])
                nc.vector.tensor_scalar(
                    exq[:, 1:2], mvq[:, 0:1], mvq[:, 0:1], mvq[:, 1:2],
                    op0=ALU.mult, op1=ALU.add,
                )
                pg = pdp.tile([8, 2], f32, tag="pd")
                nc.tensor.matmul(pg, lhsT=selg_sb, rhs=exq, start=True, stop=True)
                nc.vector.tensor_copy(out=gst[0:8, s * CT + q, :], in_=pg)
            gm = gst[0:8, s * CT : (s + 1) * CT, 0:1]
            gx2 = gst[0:8, s * CT : (s + 1) * CT, 1:2]
            # the group combine + Newton rsqrt run on Pool: DVE must stay
            # clear for the previous sample's softmax chain (D-copy/recip/
            # A-mult), which otherwise queues behind these ops and stalls PE.
            veng = nc.gpsimd
            gv = stp.tile([8, CT, 1], f32, tag="gv")
            veng.tensor_tensor(gv, gm, gm, ALU.mult)
            veng.tensor_tensor(gv, gx2, gv, ALU.subtract)
            veng.tensor_scalar(gv, gv, EPS, None, op0=ALU.add)
            # rstd = rsqrt(v): quake seed + 3 Newton steps (keeps the
            # stats chain off ACT's table-reload path; ~1e-7 rel)
            # (the int bit-trick seed ops stay on DVE: Pool lacks the
            # shift ALU op; they are 2 tiny instructions)
            i32 = dt.int32
            yb = stp.tile([8, CT, 1], f32, tag="yb")
            nc.vector.tensor_scalar(
                yb.bitcast(i32), gv.bitcast(i32), 1, None,
                op0=ALU.arith_shift_right,
            )
            nc.vector.tensor_tensor(
                yb.bitcast(i32), magic, yb.bitcast(i32), ALU.subtract
            )
            hh = stp.tile([8, CT, 1], f32, tag="hh")
            veng.tensor_scalar(hh, gv, 0.5, None, op0=ALU.mult)
            ttn = stp.tile([8, CT, 1], f32, tag="ttn")
            # 2 Newton steps: seed err ~3% -> ~1e-5 rel, far below the
            # bf16 attention interior's own rounding
            for _ in range(2):
                veng.tensor_tensor(ttn, yb, yb, ALU.mult)
                veng.tensor_tensor(ttn, hh, ttn, ALU.mult)
                veng.tensor_scalar(
                    ttn, ttn, -1.0, 1.5, op0=ALU.mult, op1=ALU.add
                )
                veng.tensor_tensor(yb, yb, ttn, ALU.mult)
            gv = yb
            # replicate each group row to its 16 channel partitions; high
            # priority so these tiny DMAs preempt bulk prefetch wire time
            # in the SP queue (they gate normalize -> w)
            rstdb = stp.tile([P, CT], f32, tag="rstdb")
            with tc.high_priority(offset=1 << 20):
                nc.sync.dma_start(
                    rstdb.opt(keep_dims={0}), pbc(gv[0:8, :, 0], 16)
                )
            gmt = stp.tile([8, CT, 1], f32, tag="gmt")
            veng.tensor_copy(out=gmt, in_=gm)
            gmb = stp.tile([P, CT], f32, tag="gmb")
            with tc.high_priority(offset=1 << 20):
                nc.sync.dma_start(
                    gmb.opt(keep_dims={0}), pbc(gmt[0:8, :, 0], 16)
                )
            cs = scv[:, s * CT : (s + 1) * CT]
            veng.tensor_tensor(cs, gw_sb, rstdb, ALU.mult)
            tmpb = stp.tile([P, CT], f32, tag="tmpb")
            veng.tensor_tensor(tmpb, gmb, cs, ALU.mult)
            veng.tensor_tensor(
                tcv[:, s * CT : (s + 1) * CT], gb_sb, tmpb, ALU.subtract
            )

        emit_stats(0)

        ptmp = ctx.enter_context(tc.tile_pool(name="ptmp", bufs=4))
        bigs = ctx.enter_context(tc.tile_pool(name="bigs", bufs=1))
        # 8-deep for the bf16 modes; the f32 fallback's 2x-size tiles need a
        # smaller pool to fit SBUF (mild A-stream pacing there is fine)
        rbp = ctx.enter_context(
            tc.tile_pool(name="rbp", bufs=8 if mdt_att is not f32 else 3)
        )

        # ---- per-sample attention ----
        nts = {}

        def emit_normalize(s):
            # on Pool (same reason as the stats chain above), EXCEPT sample 0
            # where it sits on the startup critical path and DVE is idle
            nt = bigs.tile([P, CT, HW], mdt, tag="nt", bufs=3, name=f"nt{s}")
            nts[s] = nt
            if s == 0:
                # startup critical path: 512-wide halves alternating
                # DVE/Pool so the last chunk lands in ~half the time
                for h2 in range(2 * CT):
                    q, sub = divmod(h2, 2)
                    eng = nc.vector if h2 % 2 == 0 else nc.gpsimd
                    sl = slice(sub * NF, (sub + 1) * NF)
                    eng.tensor_scalar(
                        nt[:, q, sl],
                        xts[s][:, q, sl],
                        scv[:, s * CT + q : s * CT + q + 1],
                        tcv[:, s * CT + q : s * CT + q + 1],
                        op0=ALU.mult,
                        op1=ALU.add,
                    )
                return
            for q in range(CT):
                nc.gpsimd.tensor_scalar(
                    nt[:, q],
                    xts[s][:, q],
                    scv[:, s * CT + q : s * CT + q + 1],
                    tcv[:, s * CT + q : s * CT + q + 1],
                    op0=ALU.mult,
                    op1=ALU.add,
                )

        emit_normalize(0)
        pt_sb = singles.tile([P, CT, C], mdt)
        qks, vts, ess, uss = {}, {}, {}, {}

        def w_groups(s):
            # w[c, ij] = (Wq'^T Wk') n  — the only q/k-side matmul needed:
            # scores are the Gram form S = n^T (Wq'^T Wk') n. Returned as
            # per-psum-pair closures so the caller can interleave them with
            # the previous sample's D matmuls (which pace at the exp stream).
            nt = nts[s]
            wsb = bigs.tile([P, CT, HW], mdt, tag="qk", name=f"w{s}")
            qks[s] = wsb

            def w_one(ot):
                ps = pmm.tile([P, NH, NF], f32, tag="mm")
                for n in range(NH):
                    for k in range(CT):
                        nc.tensor.matmul(
                            ps[:, n],
                            lhsT=mt_sb[:, k, ot * P : (ot + 1) * P],
                            rhs=nt[:, k, n * NF : (n + 1) * NF],
                            start=(k == 0),
                            stop=(k == CT - 1),
                        )
                nc.scalar.activation(
                    out=wsb[:, ot], in_=ps, func=AF.Identity
                )

            return [lambda ot=ot: w_one(ot) for ot in range(CT)]

        def emit_u(s):
            # u[ij] = bq'^T k'[:,ij] (the j-dependent bias term), computed
            # directly in ij-partition layout via N=2 matmuls; applied as
            # the per-partition bias of the scores exp. Emitted after the
            # D matmuls: pu shares the pdp psum pool with the D accumulators
            usb = small.tile([P, IJT], f32, tag="u", bufs=2, name=f"u{s}")
            uss[s] = usb
            nt = nts[s]
            for t in range(IJT):
                pu = pup.tile([P, 2], f32, tag="pu")
                for k in range(CT):
                    nc.tensor.matmul(
                        pu,
                        lhsT=nt[:, k, t * P : (t + 1) * P],
                        rhs=vu_sb[:, k],
                        start=(k == 0),
                        stop=(k == CT - 1),
                    )
                nc.scalar.activation(
                    out=usb[:, t : t + 1], in_=pu[:, 0:1], func=AF.Identity
                )

        def vt_groups(s):
            # n^T[ij, c] via the PE's transpose mode (the value path is
            # host-folded: attn output = (proj_w Wv) (n A), so no v
            # projection is needed on device). Same closure contract as
            # w_groups.
            nt = nts[s]
            vtsb = bigs.tile([P, IJT, C], mdt_att, tag="vt", bufs=2, name=f"ntt{s}")
            vts[s] = vtsb

            def vt_one(t2):
                # transpose output dtype must match lhsT (nt) dtype
                ps = pmm.tile([P, 2, NF], mdt, tag="mm")
                for i2 in range(2):
                    t = 2 * t2 + i2
                    for k in range(CT):
                        nc.tensor.transpose(
                            ps[:, i2, k * P : (k + 1) * P],
                            nt[:, k, t * P : (t + 1) * P],
                            ident_sb,
                        )
                # psum->sbuf (+bf16 round) on ACT: it is idle in the softmax
                # window while DVE carries the D/recip/A-mult chain
                nc.scalar.activation(
                    out=vtsb[:, 2 * t2 : 2 * t2 + 2], in_=ps, func=AF.Identity
                )

            return [lambda t2=t2: vt_one(t2) for t2 in range(IJT // 2)]

        def emit_scores_exp(s):
            # scores transposed S^T[ij, hw] = w^T n; E = exp(S^T + u[ij])
            wsb, nt, usb = qks[s], nts[s], uss[s]
            esb = bigs.tile([P, IJT, HW], mdt_att, tag="E", name=f"E{s}")
            ess[s] = esb
            for t in range(IJT):
                ps = pmm.tile([P, NH, NF], f32, tag="mm")
                for n in range(NH):
                    for k in range(CT):
                        nc.tensor.matmul(
                            ps[:, n],
                            lhsT=wsb[:, k, t * P : (t + 1) * P],
                            rhs=nt[:, k, n * NF : (n + 1) * NF],
                            start=(k == 0),
                            stop=(k == CT - 1),
                        )
                nc.scalar.activation(
                    out=esb[:, t], in_=ps, func=AF.Exp, bias=usb[:, t : t + 1]
                )

        def emit_softmax(s, fillers=()):
            # per-(i,hw) denominators D via select-matrix matmuls (sum the
            # 32 j-partitions, accumulating all 8 ij-tiles into one psum),
            # R = 1/D (custom DVE approx: quake-style fast recip straight
            # from psum + one Newton step writing the bf16 broadcast source
            # directly), then A^T = E * broadcast(R): replicate each i-row
            # of R to its 32 j-partitions with a DMA and multiply on DVE.
            # `fillers` are PE work closures (next sample's w/vt groups)
            # interleaved between D matmuls: the D stream paces at the exp
            # stream's ACT cadence, and the filler keeps the PE busy (and
            # its p-state ramped) in those gaps.
            from concourse.dve_ops import RECIPROCAL_APPROX_NR

            esb = ess[s]
            fillers = list(fillers)
            emitted = 0
            rsc = small.tile([G, HW], f32, tag="rsc")
            rrb = small.tile([G, HW], mdt_att, tag="rrb")
            for n in range(NH):
                pd = pdp.tile([P, NF], f32, tag="pd")
                for t in range(IJT):
                    nc.tensor.matmul(
                        pd,
                        lhsT=ss_sb[:, t, :],
                        rhs=esb[:, t, n * NF : (n + 1) * NF],
                        start=(t == 0),
                        stop=(t == IJT - 1),
                    )
                    done = n * IJT + t + 1
                    want = len(fillers) * done // (NH * IJT)
                    while emitted < want:
                        fillers[emitted]()
                        emitted += 1
                nc.vector.reciprocal_approx_fast(
                    out=rsc[:, n * NF : (n + 1) * NF], in_=pd[0:G]
                )
                nc.vector._custom_dve(
                    RECIPROCAL_APPROX_NR,
                    out=rrb[:, n * NF : (n + 1) * NF],
                    in0=pd[0:G],
                    in1=rsc[:, n * NF : (n + 1) * NF],
                    s0=2.0,
                )
            for f in fillers[emitted:]:
                f()
            for t in range(IJT):
                rbt = rbp.tile([P, HW], mdt_att, tag="rb")
                (nc.sync, nc.gpsimd, nc.scalar)[t % 3].dma_start(
                    rbt, pbc(rrb[4 * t : 4 * t + 4, :], 32)
                )
                # all 8 multiplies on DVE: Pool carries the next sample's
                # GN stats + normalize during this window
                nc.vector.tensor_tensor(esb[:, t], esb[:, t], rbt, ALU.mult)

        def emit_h(s):
            # h[c, hw] = sum_ij v^T[ij,c] * A^T[ij,hw]  (h overwrites nt)
            nt, vtsb, esb = nts[s], vts[s], ess[s]
            for ct in range(CT):
                ps = pmm.tile([P, NH, NF], f32, tag="mm")
                for n in range(NH):
                    for t in range(IJT):
                        nc.tensor.matmul(
                            ps[:, n],
                            lhsT=vtsb[:, t, ct * P : (ct + 1) * P],
                            rhs=esb[:, t, n * NF : (n + 1) * NF],
                            start=(t == 0),
                            stop=(t == IJT - 1),
                        )
                nc.scalar.activation(
                    out=nt[:, ct], in_=ps, func=AF.Identity
                )

        store_engs = (nc.sync, nc.scalar, nc.gpsimd)

        def proj_groups(s):
            # proj + bias + residual (accumulated into xt), then store
            nt, xt = nts[s], xts[s]
            ov = out_d[s].rearrange("(q p) f -> p q f", p=P)

            def p_one(ot):
                ps = pmm.tile([P, NH, NF], f32, tag="mm")
                for n in range(NH):
                    for k in range(CT):
                        nc.tensor.matmul(
                            ps[:, n],
                            lhsT=pt_sb[:, k, ot * P : (ot + 1) * P],
                            rhs=nt[:, k, n * NF : (n + 1) * NF],
                            start=(k == 0),
                            stop=(k == CT - 1),
                        )
                tmp = ptmp.tile([P, HW], f32, tag="pt")
                nc.scalar.activation(
                    out=tmp, in_=ps, func=AF.Identity, bias=pb_sb[:, ot : ot + 1]
                )
                nc.vector.tensor_tensor(tmp, tmp, xt[:, ot], ALU.add)
                nc.sync.dma_start(ov[:, ot], tmp)

            return [lambda ot=ot: p_one(ot) for ot in range(CT)]

        def emit_proj(s):
            for f in proj_groups(s):
                f()

        def emit_tail(s):
            # final sample: h and proj at n-half granularity so the first
            # half's proj/residual/stores overlap the second half's h
            # matmuls (h overwrites nt per half, as emit_h does wholesale)
            nt, vtsb, esb, xt = nts[s], vts[s], ess[s], xts[s]
            ov = out_d[s].rearrange("(q p) f -> p q f", p=P)
            for n in range(NH):
                sl = slice(n * NF, (n + 1) * NF)
                for ct in range(CT):
                    ps = pmm.tile([P, NF], f32, tag="mm")
                    for t in range(IJT):
                        nc.tensor.matmul(
                            ps,
                            lhsT=vtsb[:, t, ct * P : (ct + 1) * P],
                            rhs=esb[:, t, sl],
                            start=(t == 0),
                            stop=(t == IJT - 1),
                        )
                    nc.scalar.activation(
                        out=nt[:, ct, sl], in_=ps, func=AF.Identity
                    )
                for ot in range(CT):
                    ps = pmm.tile([P, NF], f32, tag="mm")
                    for k in range(CT):
                        nc.tensor.matmul(
                            ps,
                            lhsT=pt_sb[:, k, ot * P : (ot + 1) * P],
                            rhs=nt[:, k, sl],
                            start=(k == 0),
                            stop=(k == CT - 1),
                        )
                    tmp = ptmp.tile([P, NF], f32, tag="pt")
                    nc.scalar.activation(
                        out=tmp, in_=ps, func=AF.Identity,
                        bias=pb_sb[:, ot : ot + 1]
                    )
                    nc.vector.tensor_tensor(tmp, tmp, xt[:, ot, sl], ALU.add)
                    (nc.sync if ot % 2 == 0 else nc.scalar).dma_start(
                        ov[:, ot, sl], tmp
                    )

        # software pipeline: the next sample's w/vt matmul groups are
        # interleaved between this sample's D matmuls (which pace at the
        # exp stream's ACT cadence), so the PE never waits for the softmax
        # chain (D -> 1/D -> broadcast -> A-mul) to complete
        for f in w_groups(0):
            f()
        emit_u(0)
        for f in vt_groups(0):
            f()
        projected = set()
        for s in range(BS):
            emit_scores_exp(s)
            if s == 0:
                # deferred + chunked so these bulk bytes interleave with (not
                # block) the small latency-critical startup DMAs in the DMA
                # engines' queue; needed only by proj(0) much later
                ptv = pt_d.rearrange("(k p) o -> p k o", p=P)
                for ot in range(CT):
                    nc.scalar.dma_start(
                        pt_sb[:, :, ot * P : (ot + 1) * P],
                        ptv[:, :, ot * P : (ot + 1) * P],
                    )
            if s + 1 < BS:
                if s + 1 not in xts:
                    xts[s + 1] = xtp.tile(
                        [P, CT, HW], mdt_att, tag="xt", name=f"xt{s + 1}"
                    )
                    xvn = x_d[s + 1].rearrange("(q p) f -> p q f", p=P)
                    for q in range(CT):
                        nc.sync.dma_start(xts[s + 1][:, q], xvn[:, q])
                emit_stats(s + 1)
                emit_normalize(s + 1)
            emit_softmax(s)
            if s + 1 < BS:
                for f in w_groups(s + 1):
                    f()
                emit_u(s + 1)
                for f in vt_groups(s + 1):
                    f()
                if s >= 1:
                    emit_proj(s - 1)
                emit_h(s)
            else:
                emit_proj(s - 1)
                emit_tail(s)

    nc.compile()
    return nc


def _prep_inputs(x, gn_w, gn_b, qkv_w, qkv_b, proj_w, proj_b):
    x = np.asarray(x, dtype=np.float32)
    gn_w = np.asarray(gn_w, dtype=np.float32)
    gn_b = np.asarray(gn_b, dtype=np.float32)
    qkv_w = np.asarray(qkv_w, dtype=np.float32)
    qkv_b = np.asarray(qkv_b, dtype=np.float32)
    proj_w = np.asarray(proj_w, dtype=np.float32)
    proj_b = np.asarray(proj_b, dtype=np.float32)

    s4 = np.float32(float(C) ** -0.25)
    Wq = (qkv_w[:C] * s4).astype(np.float64)
    Wk = (qkv_w[C : 2 * C] * s4).astype(np.float64)
    bq = (qkv_b[:C] * s4).astype(np.float64)
    # Gram fold: S = n^T (Wq^T Wk) n + (Wk^T bq).n_ij (+ softmax-invariant
    # hw-terms, dropped). mt is the scores lhsT, vu the u-bias vector.
    mt = np.ascontiguousarray((Wk.T @ Wq).astype(np.float32))      # [C, C]
    vu = np.ascontiguousarray(np.repeat(
        (Wk.T @ bq).astype(np.float32).reshape(CT, P).T[:, :, None], 2, axis=2
    ))                                                             # [P, CT, 2]
    Wv = qkv_w[2 * C :].astype(np.float64)
    # value-path fold: attn out = (proj_w Wv) (n A); pt is the lhsT of that
    pt = np.ascontiguousarray(
        (proj_w.astype(np.float64) @ Wv).T.astype(np.float32)
    )                                                              # [C, C]
    ident = np.eye(P, dtype=np.float32)
    vb = qkv_b[2 * C :]
    pb = np.ascontiguousarray(
        (proj_b + np.float32(H) * (proj_w @ vb)).reshape(CT, P).T
    )                                                    # [P, CT]
    gw = np.ascontiguousarray(gn_w.reshape(CT, P).T)   # [P, CT]
    gb = np.ascontiguousarray(gn_b.reshape(CT, P).T)
    selg = np.zeros((P, 8), dtype=np.float32)
    selg[np.arange(P), np.arange(P) // 16] = 1.0 / 16.0
    ss = np.zeros((P, IJT, P), dtype=np.float32)
    for t in range(IJT):
        for p in range(P):
            ss[p, t, 4 * t + p // 32] = 1.0
    ss = np.ascontiguousarray(ss.reshape(P, IJT * P))
    if MM_MODE in ("f32r", "mix"):
        # the device-side "ssum" tensor is bf16 in these modes (0/1 values,
        # exact); ship matching bytes
        import ml_dtypes

        ss = ss.astype(ml_dtypes.bfloat16)
    shared = {
        "mt": mt, "vu": vu, "pt": pt, "pb": pb, "ident": ident,
        "gw": gw, "gb": gb, "ssum": ss, "selg": selg,
    }
    import ml_dtypes

    in_maps = []
    for c in range(NCORES):
        m = dict(shared)
        # x ships as bf16: frees 32KB/partition of SBUF and halves the
        # x-load wire time; costs ~4e-4 of output rel err via the GN
        # stats, normalize input, and the +x residual
        m["x"] = np.ascontiguousarray(
            x[c * BS : (c + 1) * BS].reshape(BS, C, HW)
        ).astype(ml_dtypes.bfloat16)
        in_maps.append(m)
    return in_maps


def run(inputs: dict, trace: bool = False, n_cores: int = NCORES):
    """Build (cached), run on hardware, return (results, BassKernelResults)."""
    from concourse.bass_utils import run_bass_kernel_spmd

    key = MM_MODE
    if key not in _cache:
        _cache[key] = _build(MM_MODE)
    nc = _cache[key]
    in_maps = _prep_inputs(**inputs)[:n_cores]
    res = run_bass_kernel_spmd(nc, in_maps, list(range(n_cores)), trace=trace)
    return res


def kernel(x, gn_w, gn_b, qkv_w, qkv_b, proj_w, proj_b) -> np.ndarray:
    res = run(dict(x=x, gn_w=gn_w, gn_b=gn_b, qkv_w=qkv_w, qkv_b=qkv_b,
                   proj_w=proj_w, proj_b=proj_b))
    out = np.concatenate(
        [res.results[c]["out"].reshape(BS, C, H, W) for c in range(NCORES)], axis=0
    )
    return out

